# revision 6
# baseline (speedup 1.0000x reference)
"""Trainium2 Bass kernel for nn_PositionWiseFFN_Lora (B=4, S=2048, D=1024, H=4096).

Math (see reference): per-sample LoRA outer-product updates are folded into
the dense weights on the host (W1_eff = W1 + scale*m1, W2_eff = W2 + scale*m2;
that is 0.01% of the FLOPs), leaving a pure per-sample 2-layer MLP with exact
erf-GELU:
    out[b] = gelu(x[b] @ W1_eff[b].T + b1) @ W2_eff[b].T + b2

Sharding: data-parallel over (batch, seq-half): core c handles sample c//2,
tokens [(c%2)*1024, (c%2+1)*1024). Each core runs two chained matmuls in
float32r (full-rate fp32 PE mode), hidden activations resident in SBUF.

Weights are pre-tiled on the host into the (m, p, k, j) order the PE consumes
so every DMA is a contiguous per-partition slab.
"""
import numpy as np

import concourse.bass as bass
import concourse.mybir as mybir
import concourse.tile as tile
from concourse.bass_utils import run_bass_kernel_spmd

B, S, D, H = 4, 2048, 1024, 4096
P = 128
SC = S // 2          # tokens per core
N_CORES = 8
NK1 = D // P         # 8  contraction tiles, layer 1
NM1 = H // P         # 32 output tiles, layer 1
NK2 = H // P         # 32 contraction tiles, layer 2
NM2 = D // P         # 8  output tiles, layer 2
NW = 512             # matmul moving free dim
NS = SC // NW        # 2  s-slices per core

F32 = mybir.dt.float32
F32R = mybir.dt.float32r
GELU = mybir.ActivationFunctionType.Gelu
IDENT = mybir.ActivationFunctionType.Identity

LORA_DIM = 16
LORA_TOKEN_INDEX = 1


def _legalize_sync_waits(nc):
    """The pinned walrus supports at most 2 sem waits per instruction (0 for
    InstDrain). Move excess waits onto freshly inserted EventSemaphore insts
    (<=2 waits each) right before the instruction on the same engine."""
    n_fixed = 0
    for fn in nc.m.functions:
        for bb in fn.blocks:
            insts = bb.instructions
            i = 0
            while i < len(insts):
                inst = insts[i]
                si = inst.sync_info
                if si is None or not si.on_wait:
                    i += 1
                    continue
                if isinstance(inst, mybir.InstDrain):
                    cap = 0
                elif isinstance(inst, mybir.InstEventSemaphore):
                    cap = 2
                else:
                    cap = 1
                if len(si.on_wait) <= cap:
                    i += 1
                    continue
                waits = list(si.on_wait)
                keep = waits[:cap]
                move = waits[cap:]
                for j in range(0, len(move), 2):
                    ev = mybir.InstEventSemaphore(
                        name=f"lgw_{inst.name}_{j // 2}", ins=[], outs=[]
                    )
                    ev.engine = inst.engine
                    ev.sync_info = mybir.SyncInfo(on_wait=move[j : j + 2], on_update=[])
                    ev.bass_nofuse = True
                    nc.register_instruction(ev)
                    insts.insert(i, ev)
                    i += 1
                si.on_wait = keep
                n_fixed += 1
                i += 1
    return n_fixed


def build_program():
    """Build the per-core Bass program (identical on all 8 cores)."""
    nc = bass.Bass(
        trn_type="TRN2", target_bir_lowering=False, debug=False, num_devices=N_CORES
    )
    xt_d = nc.dram_tensor("xt", [D, SC], F32R, kind="ExternalInput")
    a1_d = nc.dram_tensor("a1", [NM1, P, NK1 * P], F32R, kind="ExternalInput")
    a2_d = nc.dram_tensor("a2", [NM2, P, NK2 * P], F32R, kind="ExternalInput")
    b1_d = nc.dram_tensor("b1t", [P, NM1], F32, kind="ExternalInput")
    b2_d = nc.dram_tensor("b2t", [P, NM2], F32, kind="ExternalInput")
    out_d = nc.dram_tensor("out", [D, SC], F32, kind="ExternalOutput")

    xt = xt_d.ap()
    a1 = a1_d.ap()
    a2 = a2_d.ap()
    out = out_d.ap()

    with tile.TileContext(nc) as tc:
        with (
            tc.tile_pool(name="xp", bufs=1) as xpool,
            tc.tile_pool(name="hp", bufs=1) as hpool,
            tc.tile_pool(name="w1p", bufs=2) as w1pool,
            tc.tile_pool(name="w2p", bufs=2) as w2pool,
            tc.tile_pool(name="op", bufs=2) as opool,
            tc.tile_pool(name="cp", bufs=1) as cpool,
            tc.tile_pool(name="psp", bufs=2, space="PSUM") as pspool,
        ):
            b1s = cpool.tile([P, NM1], F32, tag="b1s")
            nc.sync.dma_start(out=b1s[:], in_=b1_d.ap())
            b2s = cpool.tile([P, NM2], F32, tag="b2s")
            nc.sync.dma_start(out=b2s[:], in_=b2_d.ap())

            xts = []
            for k in range(NK1):
                xk = xpool.tile([P, SC], F32R, tag=f"x{k}", name=f"x{k}")
                nc.sync.dma_start(out=xk[:], in_=xt[k * P : (k + 1) * P, :])
                xts.append(xk)

            # ---- layer 1: hT[m] = gelu(W1_eff @ x^T + b1), (128h, SC s) ----
            hts = []
            for m in range(NM1):
                w1m = w1pool.tile([P, NK1 * P], F32R, tag="w1m", name=f"w1m{m}")
                nc.sync.dma_start(out=w1m[:], in_=a1[m])
                pss = [
                    pspool.tile([P, NW], F32, tag=f"ps{si}", name=f"ps{si}_{m}")
                    for si in range(NS)
                ]
                for k in range(NK1):
                    lhsT = w1m[:, k * P : (k + 1) * P]
                    for si in range(NS):
                        nc.tensor.matmul(
                            pss[si][:],
                            lhsT=lhsT,
                            rhs=xts[k][:, si * NW : (si + 1) * NW],
                            start=(k == 0),
                            stop=(k == NK1 - 1),
                        )
                hm = hpool.tile([P, SC], F32R, tag=f"h{m}", name=f"h{m}")
                for si in range(NS):
                    nc.scalar.activation(
                        hm[:, si * NW : (si + 1) * NW],
                        pss[si][:],
                        GELU,
                        bias=b1s[:, m : m + 1],
                    )
                hts.append(hm)

            # ---- layer 2: outT[m] = W2_eff @ gelu_hT + b2, (128o, SC s) ----
            KC = 8          # k-tiles per streamed weight chunk
            for m in range(NM2):
                qss = [
                    pspool.tile([P, NW], F32, tag=f"q{si}", name=f"q{si}_{m}")
                    for si in range(NS)
                ]
                for kc in range(NK2 // KC):
                    w2c = w2pool.tile(
                        [P, KC * P], F32R, tag="w2c", name=f"w2c{m}_{kc}"
                    )
                    nc.sync.dma_start(
                        out=w2c[:], in_=a2[m, :, kc * KC * P : (kc + 1) * KC * P]
                    )
                    for kk in range(KC):
                        k = kc * KC + kk
                        lhsT = w2c[:, kk * P : (kk + 1) * P]
                        for si in range(NS):
                            nc.tensor.matmul(
                                qss[si][:],
                                lhsT=lhsT,
                                rhs=hts[k][:, si * NW : (si + 1) * NW],
                                start=(k == 0),
                                stop=(k == NK2 - 1),
                            )
                for si in range(NS):
                    om = opool.tile([P, NW], F32, tag=f"o{si}", name=f"o{si}_{m}")
                    nc.scalar.activation(
                        om[:], qss[si][:], IDENT, bias=b2s[:, m : m + 1]
                    )
                    nc.sync.dma_start(
                        out=out[m * P : (m + 1) * P, si * NW : (si + 1) * NW],
                        in_=om[:],
                    )

    _legalize_sync_waits(nc)
    return nc


def prepare_core_inputs(x, W1, b1, W2, b2, scale):
    """Host-side: fold LoRA into weights, tile for the PE, shard per core."""
    x = np.asarray(x, dtype=np.float32)
    W1 = np.asarray(W1, dtype=np.float32)
    W2 = np.asarray(W2, dtype=np.float32)
    b1 = np.asarray(b1, dtype=np.float32)
    b2 = np.asarray(b2, dtype=np.float32)
    scale = np.float32(np.asarray(scale).reshape(()))

    lt = x[:, LORA_TOKEN_INDEX : LORA_TOKEN_INDEX + LORA_DIM, :]
    even = lt[:, ::2, :]  # (B, 8, D)
    odd = lt[:, 1::2, :]  # (B, 8, D)
    m1 = (even[:, :4, :, None] * odd[:, :4, None, :]).reshape(B, H, D)
    m2 = (even[:, 4:, :, None] * odd[:, 4:, None, :]).reshape(B, D, H)
    W1e = W1[None, :, :] + scale * m1  # (B, H, D)
    W2e = W2[None, :, :] + scale * m2  # (B, D, H)

    # lhsT tiles, DMA-contiguous: A1[b][m,p,k,j] = W1e[b, m*128+j, k*128+p]
    A1 = np.ascontiguousarray(
        W1e.reshape(B, NM1, P, NK1, P).transpose(0, 1, 4, 3, 2)
    ).reshape(B, NM1, P, NK1 * P)
    A2 = np.ascontiguousarray(
        W2e.reshape(B, NM2, P, NK2, P).transpose(0, 1, 4, 3, 2)
    ).reshape(B, NM2, P, NK2 * P)
    b1t = np.ascontiguousarray(b1.reshape(NM1, P).T)  # (128, 32)
    b2t = np.ascontiguousarray(b2.reshape(NM2, P).T)  # (128, 8)

    in_maps = []
    for c in range(N_CORES):
        b, r = divmod(c, 2)
        xs = x[b, r * SC : (r + 1) * SC, :]  # (SC, D)
        in_maps.append(
            {
                "xt": np.ascontiguousarray(xs.T),  # (D, SC)
                "a1": A1[b],
                "a2": A2[b],
                "b1t": b1t,
                "b2t": b2t,
            }
        )
    return in_maps


_PROGRAM_CACHE = {}


def get_program():
    if "nc" not in _PROGRAM_CACHE:
        _PROGRAM_CACHE["nc"] = build_program()
    return _PROGRAM_CACHE["nc"]


def kernel(x, W1, b1, W2, b2, scale, _run_kwargs=None):
    nc = get_program()
    in_maps = prepare_core_inputs(x, W1, b1, W2, b2, scale)
    res = run_bass_kernel_spmd(
        nc, in_maps, list(range(N_CORES)), **(_run_kwargs or {})
    )
    out = np.empty((B, S, D), dtype=np.float32)
    for c in range(N_CORES):
        b, r = divmod(c, 2)
        out[b, r * SC : (r + 1) * SC, :] = res.results[c]["out"].T
    if _run_kwargs:
        kernel.last_result = res
    return out


# revision 8
# speedup vs baseline: 1.1017x; 1.1017x over previous
"""Trainium2 Bass kernel for nn_PositionWiseFFN_Lora (B=4, S=2048, D=1024, H=4096).

Math (see reference): per-sample LoRA outer-product updates are folded into
the dense weights on the host (W1_eff = W1 + scale*m1, W2_eff = W2 + scale*m2;
that is 0.01% of the FLOPs), leaving a pure per-sample 2-layer MLP with exact
erf-GELU:
    out[b] = gelu(x[b] @ W1_eff[b].T + b1) @ W2_eff[b].T + b2

Sharding: data-parallel over (batch, seq-half): core c handles sample c//2,
tokens [(c%2)*1024, (c%2+1)*1024). Each core runs two chained matmuls in
float32r (full-rate fp32 PE mode), hidden activations resident in SBUF.

Weights are pre-tiled on the host into the (m, p, k, j) order the PE consumes
so every DMA is a contiguous per-partition slab.
"""
import numpy as np

import concourse.bass as bass
import concourse.mybir as mybir
import concourse.tile as tile
from concourse.bass_utils import run_bass_kernel_spmd

B, S, D, H = 4, 2048, 1024, 4096
P = 128
SC = S // 2          # tokens per core
N_CORES = 8
NK1 = D // P         # 8  contraction tiles, layer 1
NM1 = H // P         # 32 output tiles, layer 1
NK2 = H // P         # 32 contraction tiles, layer 2
NM2 = D // P         # 8  output tiles, layer 2
NW = 512             # matmul moving free dim
NS = SC // NW        # 2  s-slices per core

F32 = mybir.dt.float32
F32R = mybir.dt.float32r
GELU = mybir.ActivationFunctionType.Gelu
IDENT = mybir.ActivationFunctionType.Identity

LORA_DIM = 16
LORA_TOKEN_INDEX = 1


def _legalize_sync_waits(nc):
    """The pinned walrus supports at most 2 sem waits per instruction (0 for
    InstDrain). Move excess waits onto freshly inserted EventSemaphore insts
    (<=2 waits each) right before the instruction on the same engine."""
    n_fixed = 0
    for fn in nc.m.functions:
        for bb in fn.blocks:
            insts = bb.instructions
            i = 0
            while i < len(insts):
                inst = insts[i]
                si = inst.sync_info
                if si is None or not si.on_wait:
                    i += 1
                    continue
                if isinstance(inst, mybir.InstDrain):
                    cap = 0
                elif isinstance(inst, mybir.InstEventSemaphore):
                    cap = 2
                else:
                    cap = 1
                if len(si.on_wait) <= cap:
                    i += 1
                    continue
                waits = list(si.on_wait)
                keep = waits[:cap]
                move = waits[cap:]
                for j in range(0, len(move), 2):
                    ev = mybir.InstEventSemaphore(
                        name=f"lgw_{inst.name}_{j // 2}", ins=[], outs=[]
                    )
                    ev.engine = inst.engine
                    ev.sync_info = mybir.SyncInfo(on_wait=move[j : j + 2], on_update=[])
                    ev.bass_nofuse = True
                    nc.register_instruction(ev)
                    insts.insert(i, ev)
                    i += 1
                si.on_wait = keep
                n_fixed += 1
                i += 1
    return n_fixed


def build_program():
    """Build the per-core Bass program (identical on all 8 cores)."""
    nc = bass.Bass(
        trn_type="TRN2", target_bir_lowering=False, debug=False, num_devices=N_CORES
    )
    xt_d = nc.dram_tensor("xt", [D, SC], F32R, kind="ExternalInput")
    a1_d = nc.dram_tensor("a1", [NM1, P, NK1 * P], F32R, kind="ExternalInput")
    a2_d = nc.dram_tensor("a2", [NM2, P, NK2 * P], F32R, kind="ExternalInput")
    b1_d = nc.dram_tensor("b1t", [P, NM1], F32, kind="ExternalInput")
    b2_d = nc.dram_tensor("b2t", [P, NM2], F32, kind="ExternalInput")
    out_d = nc.dram_tensor("out", [D, SC], F32, kind="ExternalOutput")

    xt = xt_d.ap()
    a1 = a1_d.ap()
    a2 = a2_d.ap()
    out = out_d.ap()

    with tile.TileContext(nc) as tc:
        with (
            tc.tile_pool(name="xp", bufs=1) as xpool,
            tc.tile_pool(name="hp", bufs=1) as hpool,
            tc.tile_pool(name="w1p", bufs=3) as w1pool,
            tc.tile_pool(name="w2p", bufs=4) as w2pool,
            tc.tile_pool(name="op", bufs=2) as opool,
            tc.tile_pool(name="cp", bufs=1) as cpool,
            tc.tile_pool(name="psp", bufs=2, space="PSUM") as pspool,
        ):
            b1s = cpool.tile([P, NM1], F32, tag="b1s")
            nc.sync.dma_start(out=b1s[:], in_=b1_d.ap())
            b2s = cpool.tile([P, NM2], F32, tag="b2s")
            nc.sync.dma_start(out=b2s[:], in_=b2_d.ap())

            # load x si-half-major so the first matmuls' deps land early
            xts = [
                xpool.tile([P, SC], F32R, tag=f"x{k}", name=f"x{k}")
                for k in range(NK1)
            ]
            for si in range(NS):
                for k in range(NK1):
                    nc.sync.dma_start(
                        out=xts[k][:, si * NW : (si + 1) * NW],
                        in_=xt[k * P : (k + 1) * P, si * NW : (si + 1) * NW],
                    )

            # ---- layer 1: hT[m] = gelu(W1_eff @ x^T + b1), (128h, SC s) ----
            hts = []
            for m in range(NM1):
                w1m = w1pool.tile([P, NK1 * P], F32R, tag="w1m", name=f"w1m{m}")
                nc.sync.dma_start(out=w1m[:], in_=a1[m])
                hm = hpool.tile([P, SC], F32R, tag=f"h{m}", name=f"h{m}")
                for si in range(NS):
                    ps = pspool.tile([P, NW], F32, tag=f"ps{si}", name=f"ps{si}_{m}")
                    for k in range(NK1):
                        nc.tensor.matmul(
                            ps[:],
                            lhsT=w1m[:, k * P : (k + 1) * P],
                            rhs=xts[k][:, si * NW : (si + 1) * NW],
                            start=(k == 0),
                            stop=(k == NK1 - 1),
                        )
                    nc.scalar.activation(
                        hm[:, si * NW : (si + 1) * NW],
                        ps[:],
                        GELU,
                        bias=b1s[:, m : m + 1],
                    )
                hts.append(hm)

            # ---- layer 2: outT[m] = W2_eff @ gelu_hT + b2, (128o, SC s) ----
            KC = 8          # k-tiles per streamed weight chunk
            for m in range(NM2):
                qss = [
                    pspool.tile([P, NW], F32, tag=f"q{si}", name=f"q{si}_{m}")
                    for si in range(NS)
                ]
                for kc in range(NK2 // KC):
                    w2c = w2pool.tile(
                        [P, KC * P], F32R, tag="w2c", name=f"w2c{m}_{kc}"
                    )
                    nc.sync.dma_start(
                        out=w2c[:], in_=a2[m, :, kc * KC * P : (kc + 1) * KC * P]
                    )
                    for kk in range(KC):
                        k = kc * KC + kk
                        lhsT = w2c[:, kk * P : (kk + 1) * P]
                        for si in range(NS):
                            nc.tensor.matmul(
                                qss[si][:],
                                lhsT=lhsT,
                                rhs=hts[k][:, si * NW : (si + 1) * NW],
                                start=(k == 0),
                                stop=(k == NK2 - 1),
                            )
                for si in range(NS):
                    om = opool.tile([P, NW], F32, tag=f"o{si}", name=f"o{si}_{m}")
                    nc.scalar.activation(
                        om[:], qss[si][:], IDENT, bias=b2s[:, m : m + 1]
                    )
                    nc.sync.dma_start(
                        out=out[m * P : (m + 1) * P, si * NW : (si + 1) * NW],
                        in_=om[:],
                    )

    _legalize_sync_waits(nc)
    return nc


def prepare_core_inputs(x, W1, b1, W2, b2, scale):
    """Host-side: fold LoRA into weights, tile for the PE, shard per core."""
    x = np.asarray(x, dtype=np.float32)
    W1 = np.asarray(W1, dtype=np.float32)
    W2 = np.asarray(W2, dtype=np.float32)
    b1 = np.asarray(b1, dtype=np.float32)
    b2 = np.asarray(b2, dtype=np.float32)
    scale = np.float32(np.asarray(scale).reshape(()))

    lt = x[:, LORA_TOKEN_INDEX : LORA_TOKEN_INDEX + LORA_DIM, :]
    even = lt[:, ::2, :]  # (B, 8, D)
    odd = lt[:, 1::2, :]  # (B, 8, D)
    m1 = (even[:, :4, :, None] * odd[:, :4, None, :]).reshape(B, H, D)
    m2 = (even[:, 4:, :, None] * odd[:, 4:, None, :]).reshape(B, D, H)
    W1e = W1[None, :, :] + scale * m1  # (B, H, D)
    W2e = W2[None, :, :] + scale * m2  # (B, D, H)

    # lhsT tiles, DMA-contiguous: A1[b][m,p,k,j] = W1e[b, m*128+j, k*128+p]
    A1 = np.ascontiguousarray(
        W1e.reshape(B, NM1, P, NK1, P).transpose(0, 1, 4, 3, 2)
    ).reshape(B, NM1, P, NK1 * P)
    A2 = np.ascontiguousarray(
        W2e.reshape(B, NM2, P, NK2, P).transpose(0, 1, 4, 3, 2)
    ).reshape(B, NM2, P, NK2 * P)
    b1t = np.ascontiguousarray(b1.reshape(NM1, P).T)  # (128, 32)
    b2t = np.ascontiguousarray(b2.reshape(NM2, P).T)  # (128, 8)

    in_maps = []
    for c in range(N_CORES):
        b, r = divmod(c, 2)
        xs = x[b, r * SC : (r + 1) * SC, :]  # (SC, D)
        in_maps.append(
            {
                "xt": np.ascontiguousarray(xs.T),  # (D, SC)
                "a1": A1[b],
                "a2": A2[b],
                "b1t": b1t,
                "b2t": b2t,
            }
        )
    return in_maps


_PROGRAM_CACHE = {}


def get_program():
    if "nc" not in _PROGRAM_CACHE:
        _PROGRAM_CACHE["nc"] = build_program()
    return _PROGRAM_CACHE["nc"]


def kernel(x, W1, b1, W2, b2, scale, _run_kwargs=None):
    nc = get_program()
    in_maps = prepare_core_inputs(x, W1, b1, W2, b2, scale)
    res = run_bass_kernel_spmd(
        nc, in_maps, list(range(N_CORES)), **(_run_kwargs or {})
    )
    out = np.empty((B, S, D), dtype=np.float32)
    for c in range(N_CORES):
        b, r = divmod(c, 2)
        out[b, r * SC : (r + 1) * SC, :] = res.results[c]["out"].T
    if _run_kwargs:
        kernel.last_result = res
    return out


# revision 12
# speedup vs baseline: 1.1051x; 1.0031x over previous
"""Trainium2 Bass kernel for nn_PositionWiseFFN_Lora (B=4, S=2048, D=1024, H=4096).

Math (see reference): per-sample LoRA outer-product updates are folded into
the dense weights on the host (W1_eff = W1 + scale*m1, W2_eff = W2 + scale*m2;
that is 0.01% of the FLOPs), leaving a pure per-sample 2-layer MLP with exact
erf-GELU:
    out[b] = gelu(x[b] @ W1_eff[b].T + b1) @ W2_eff[b].T + b2

Sharding: data-parallel over (batch, seq-half): core c handles sample c//2,
tokens [(c%2)*1024, (c%2+1)*1024). Each core runs two chained matmuls in
float32r (full-rate fp32 PE mode), hidden activations resident in SBUF.

Weights are pre-tiled on the host into the (m, p, k, j) order the PE consumes
so every DMA is a contiguous per-partition slab.
"""
import numpy as np

import concourse.bass as bass
import concourse.mybir as mybir
import concourse.tile as tile
from concourse.bass_utils import run_bass_kernel_spmd

B, S, D, H = 4, 2048, 1024, 4096
P = 128
SC = S // 2          # tokens per core
N_CORES = 8
NK1 = D // P         # 8  contraction tiles, layer 1
NM1 = H // P         # 32 output tiles, layer 1
NK2 = H // P         # 32 contraction tiles, layer 2
NM2 = D // P         # 8  output tiles, layer 2
NW = 512             # matmul moving free dim
NS = SC // NW        # 2  s-slices per core

F32 = mybir.dt.float32
F32R = mybir.dt.float32r
GELU = mybir.ActivationFunctionType.Gelu
IDENT = mybir.ActivationFunctionType.Identity

LORA_DIM = 16
LORA_TOKEN_INDEX = 1


def _legalize_sync_waits(nc):
    """The pinned walrus supports at most 2 sem waits per instruction (0 for
    InstDrain). Move excess waits onto freshly inserted EventSemaphore insts
    (<=2 waits each) right before the instruction on the same engine."""
    n_fixed = 0
    for fn in nc.m.functions:
        for bb in fn.blocks:
            insts = bb.instructions
            i = 0
            while i < len(insts):
                inst = insts[i]
                si = inst.sync_info
                if si is None or not si.on_wait:
                    i += 1
                    continue
                if isinstance(inst, mybir.InstDrain):
                    cap = 0
                elif isinstance(inst, mybir.InstEventSemaphore):
                    cap = 2
                else:
                    cap = 1
                if len(si.on_wait) <= cap:
                    i += 1
                    continue
                waits = list(si.on_wait)
                keep = waits[:cap]
                move = waits[cap:]
                for j in range(0, len(move), 2):
                    ev = mybir.InstEventSemaphore(
                        name=f"lgw_{inst.name}_{j // 2}", ins=[], outs=[]
                    )
                    ev.engine = inst.engine
                    ev.sync_info = mybir.SyncInfo(on_wait=move[j : j + 2], on_update=[])
                    ev.bass_nofuse = True
                    nc.register_instruction(ev)
                    insts.insert(i, ev)
                    i += 1
                si.on_wait = keep
                n_fixed += 1
                i += 1
    return n_fixed


def build_program():
    """Build the per-core Bass program (identical on all 8 cores)."""
    nc = bass.Bass(
        trn_type="TRN2", target_bir_lowering=False, debug=False, num_devices=N_CORES
    )
    xt_d = nc.dram_tensor("xt", [D, SC], F32R, kind="ExternalInput")
    a1_d = nc.dram_tensor("a1", [NM1, P, NK1 * P], F32R, kind="ExternalInput")
    a2_d = nc.dram_tensor("a2", [NM2, P, NK2 * P], F32R, kind="ExternalInput")
    b1_d = nc.dram_tensor("b1t", [P, NM1], F32, kind="ExternalInput")
    b2_d = nc.dram_tensor("b2t", [P, NM2], F32, kind="ExternalInput")
    out_d = nc.dram_tensor("out", [D, SC], F32, kind="ExternalOutput")

    xt = xt_d.ap()
    a1 = a1_d.ap()
    a2 = a2_d.ap()
    out = out_d.ap()

    with tile.TileContext(nc) as tc:
        with (
            tc.tile_pool(name="xp", bufs=1) as xpool,
            tc.tile_pool(name="hp", bufs=1) as hpool,
            tc.tile_pool(name="w1p", bufs=3) as w1pool,
            tc.tile_pool(name="w2p", bufs=4) as w2pool,
            tc.tile_pool(name="op", bufs=2) as opool,
            tc.tile_pool(name="cp", bufs=1) as cpool,
            tc.tile_pool(name="psp", bufs=2, space="PSUM") as pspool,
        ):
            # x folded to one tile per s-half: xs[si][p, k*NW+s] = xt[k*P+p, si*NW+s]
            # (single DMA each; si=0 lands first so matmuls start early)
            xsrc = xt.rearrange("(k p) s -> p k s", p=P)  # [p, k, s]
            xs = []
            for si in range(NS):
                xh = xpool.tile([P, NK1 * NW], F32R, tag=f"xs{si}", name=f"xs{si}")
                nc.sync.dma_start(
                    out=xh.rearrange("p (k s) -> p k s", s=NW),
                    in_=xsrc[:, :, si * NW : (si + 1) * NW],
                )
                xs.append(xh)

            b1s = cpool.tile([P, NM1], F32, tag="b1s")
            nc.sync.dma_start(out=b1s[:], in_=b1_d.ap())
            b2s = cpool.tile([P, NM2], F32, tag="b2s")
            nc.sync.dma_start(out=b2s[:], in_=b2_d.ap())

            # ---- layer 1: hT[m] = gelu(W1_eff @ x^T + b1), (128h, SC s) ----
            hts = []
            for m in range(NM1):
                w1m = w1pool.tile([P, NK1 * P], F32R, tag="w1m", name=f"w1m{m}")
                nc.sync.dma_start(out=w1m[:], in_=a1[m])
                hm = hpool.tile([P, SC], F32R, tag=f"h{m}", name=f"h{m}")
                for si in range(NS):
                    ps = pspool.tile([P, NW], F32, tag=f"ps{si}", name=f"ps{si}_{m}")
                    for k in range(NK1):
                        nc.tensor.matmul(
                            ps[:],
                            lhsT=w1m[:, k * P : (k + 1) * P],
                            rhs=xs[si][:, k * NW : (k + 1) * NW],
                            start=(k == 0),
                            stop=(k == NK1 - 1),
                        )
                    nc.scalar.activation(
                        hm[:, si * NW : (si + 1) * NW],
                        ps[:],
                        GELU,
                        bias=b1s[:, m : m + 1],
                    )
                hts.append(hm)

            # ---- layer 2: outT[m] = W2_eff @ gelu_hT + b2, (128o, SC s) ----
            KC = 8          # k-tiles per streamed weight chunk
            for m in range(NM2):
                qss = [
                    pspool.tile([P, NW], F32, tag=f"q{si}", name=f"q{si}_{m}")
                    for si in range(NS)
                ]
                for kc in range(NK2 // KC):
                    w2c = w2pool.tile(
                        [P, KC * P], F32R, tag="w2c", name=f"w2c{m}_{kc}"
                    )
                    nc.sync.dma_start(
                        out=w2c[:], in_=a2[m, :, kc * KC * P : (kc + 1) * KC * P]
                    )
                    for kk in range(KC):
                        k = kc * KC + kk
                        lhsT = w2c[:, kk * P : (kk + 1) * P]
                        for si in range(NS):
                            nc.tensor.matmul(
                                qss[si][:],
                                lhsT=lhsT,
                                rhs=hts[k][:, si * NW : (si + 1) * NW],
                                start=(k == 0),
                                stop=(k == NK2 - 1),
                            )
                for si in range(NS):
                    om = opool.tile([P, NW], F32, tag=f"o{si}", name=f"o{si}_{m}")
                    nc.scalar.activation(
                        om[:], qss[si][:], IDENT, bias=b2s[:, m : m + 1]
                    )
                    nc.sync.dma_start(
                        out=out[m * P : (m + 1) * P, si * NW : (si + 1) * NW],
                        in_=om[:],
                    )

    _legalize_sync_waits(nc)
    return nc


def prepare_core_inputs(x, W1, b1, W2, b2, scale):
    """Host-side: fold LoRA into weights, tile for the PE, shard per core."""
    x = np.asarray(x, dtype=np.float32)
    W1 = np.asarray(W1, dtype=np.float32)
    W2 = np.asarray(W2, dtype=np.float32)
    b1 = np.asarray(b1, dtype=np.float32)
    b2 = np.asarray(b2, dtype=np.float32)
    scale = np.float32(np.asarray(scale).reshape(()))

    lt = x[:, LORA_TOKEN_INDEX : LORA_TOKEN_INDEX + LORA_DIM, :]
    even = lt[:, ::2, :]  # (B, 8, D)
    odd = lt[:, 1::2, :]  # (B, 8, D)
    m1 = (even[:, :4, :, None] * odd[:, :4, None, :]).reshape(B, H, D)
    m2 = (even[:, 4:, :, None] * odd[:, 4:, None, :]).reshape(B, D, H)
    W1e = W1[None, :, :] + scale * m1  # (B, H, D)
    W2e = W2[None, :, :] + scale * m2  # (B, D, H)

    # lhsT tiles, DMA-contiguous: A1[b][m,p,k,j] = W1e[b, m*128+j, k*128+p]
    A1 = np.ascontiguousarray(
        W1e.reshape(B, NM1, P, NK1, P).transpose(0, 1, 4, 3, 2)
    ).reshape(B, NM1, P, NK1 * P)
    A2 = np.ascontiguousarray(
        W2e.reshape(B, NM2, P, NK2, P).transpose(0, 1, 4, 3, 2)
    ).reshape(B, NM2, P, NK2 * P)
    b1t = np.ascontiguousarray(b1.reshape(NM1, P).T)  # (128, 32)
    b2t = np.ascontiguousarray(b2.reshape(NM2, P).T)  # (128, 8)

    in_maps = []
    for c in range(N_CORES):
        b, r = divmod(c, 2)
        xs = x[b, r * SC : (r + 1) * SC, :]  # (SC, D)
        in_maps.append(
            {
                "xt": np.ascontiguousarray(xs.T),  # (D, SC)
                "a1": A1[b],
                "a2": A2[b],
                "b1t": b1t,
                "b2t": b2t,
            }
        )
    return in_maps


_PROGRAM_CACHE = {}


def get_program():
    if "nc" not in _PROGRAM_CACHE:
        _PROGRAM_CACHE["nc"] = build_program()
    return _PROGRAM_CACHE["nc"]


def kernel(x, W1, b1, W2, b2, scale, _run_kwargs=None):
    nc = get_program()
    in_maps = prepare_core_inputs(x, W1, b1, W2, b2, scale)
    res = run_bass_kernel_spmd(
        nc, in_maps, list(range(N_CORES)), **(_run_kwargs or {})
    )
    out = np.empty((B, S, D), dtype=np.float32)
    for c in range(N_CORES):
        b, r = divmod(c, 2)
        out[b, r * SC : (r + 1) * SC, :] = res.results[c]["out"].T
    if _run_kwargs:
        kernel.last_result = res
    return out


# revision 15
# speedup vs baseline: 1.1359x; 1.0279x over previous
"""Trainium2 Bass kernel for nn_PositionWiseFFN_Lora (B=4, S=2048, D=1024, H=4096).

Math (see reference): per-sample LoRA outer-product updates are folded into
the dense weights on the host (W1_eff = W1 + scale*m1, W2_eff = W2 + scale*m2;
that is 0.01% of the FLOPs), leaving a pure per-sample 2-layer MLP with exact
erf-GELU:
    out[b] = gelu(x[b] @ W1_eff[b].T + b1) @ W2_eff[b].T + b2

Sharding: data-parallel over (batch, seq-half): core c handles sample c//2,
tokens [(c%2)*1024, (c%2+1)*1024). Each core runs two chained matmuls in
float32r (full-rate fp32 PE mode), hidden activations resident in SBUF.

Weights are pre-tiled on the host into the (m, p, k, j) order the PE consumes
so every DMA is a contiguous per-partition slab.
"""
import numpy as np

import concourse.bass as bass
import concourse.mybir as mybir
import concourse.tile as tile
from concourse.bass_utils import run_bass_kernel_spmd
from concourse.tile import add_dep_helper

B, S, D, H = 4, 2048, 1024, 4096
P = 128
SC = S // 2          # tokens per core
N_CORES = 8
NK1 = D // P         # 8  contraction tiles, layer 1
NM1 = H // P         # 32 output tiles, layer 1
NK2 = H // P         # 32 contraction tiles, layer 2
NM2 = D // P         # 8  output tiles, layer 2
NW = 512             # matmul moving free dim
NS = SC // NW        # 2  s-slices per core

F32 = mybir.dt.float32
F32R = mybir.dt.float32r
GELU = mybir.ActivationFunctionType.Gelu
IDENT = mybir.ActivationFunctionType.Identity

LORA_DIM = 16
LORA_TOKEN_INDEX = 1


def _legalize_sync_waits(nc):
    """The pinned walrus supports at most 2 sem waits per instruction (0 for
    InstDrain). Move excess waits onto freshly inserted EventSemaphore insts
    (<=2 waits each) right before the instruction on the same engine."""
    n_fixed = 0
    for fn in nc.m.functions:
        for bb in fn.blocks:
            insts = bb.instructions
            i = 0
            while i < len(insts):
                inst = insts[i]
                si = inst.sync_info
                if si is None or not si.on_wait:
                    i += 1
                    continue
                if isinstance(inst, mybir.InstDrain):
                    cap = 0
                elif isinstance(inst, mybir.InstEventSemaphore):
                    cap = 2
                else:
                    cap = 1
                if len(si.on_wait) <= cap:
                    i += 1
                    continue
                waits = list(si.on_wait)
                keep = waits[:cap]
                move = waits[cap:]
                for j in range(0, len(move), 2):
                    ev = mybir.InstEventSemaphore(
                        name=f"lgw_{inst.name}_{j // 2}", ins=[], outs=[]
                    )
                    ev.engine = inst.engine
                    ev.sync_info = mybir.SyncInfo(on_wait=move[j : j + 2], on_update=[])
                    ev.bass_nofuse = True
                    nc.register_instruction(ev)
                    insts.insert(i, ev)
                    i += 1
                si.on_wait = keep
                n_fixed += 1
                i += 1
    return n_fixed


def build_program():
    """Build the per-core Bass program (identical on all 8 cores)."""
    nc = bass.Bass(
        trn_type="TRN2", target_bir_lowering=False, debug=False, num_devices=N_CORES
    )
    xt_d = nc.dram_tensor("xt", [D, SC], F32R, kind="ExternalInput")
    a1_d = nc.dram_tensor("a1", [NM1, P, NK1 * P], F32R, kind="ExternalInput")
    a2_d = nc.dram_tensor("a2", [NM2, P, NK2 * P], F32R, kind="ExternalInput")
    b1_d = nc.dram_tensor("b1t", [P, NM1], F32, kind="ExternalInput")
    b2_d = nc.dram_tensor("b2t", [P, NM2], F32, kind="ExternalInput")
    out_d = nc.dram_tensor("out", [D, SC], F32, kind="ExternalOutput")

    xt = xt_d.ap()
    a1 = a1_d.ap()
    a2 = a2_d.ap()
    out = out_d.ap()

    with tile.TileContext(nc) as tc:
        with (
            tc.tile_pool(name="xp", bufs=1) as xpool,
            tc.tile_pool(name="hp", bufs=1) as hpool,
            tc.tile_pool(name="w1p", bufs=3) as w1pool,
            tc.tile_pool(name="w2p", bufs=4) as w2pool,
            tc.tile_pool(name="op", bufs=2) as opool,
            tc.tile_pool(name="cp", bufs=1) as cpool,
            tc.tile_pool(name="psp", bufs=2, space="PSUM") as pspool,
        ):
            # x per-k tiles on the gpsimd DMA queue: issue stream parallel to
            # the weight DMAs on sync, and matmuls start as soon as x0 lands
            xts = []
            for k in range(NK1):
                xk = xpool.tile([P, SC], F32R, tag=f"x{k}", name=f"x{k}")
                nc.gpsimd.dma_start(out=xk[:], in_=xt[k * P : (k + 1) * P, :])
                xts.append(xk)

            b1s = cpool.tile([P, NM1], F32, tag="b1s")
            nc.gpsimd.dma_start(out=b1s[:], in_=b1_d.ap())
            b2s = cpool.tile([P, NM2], F32, tag="b2s")
            nc.gpsimd.dma_start(out=b2s[:], in_=b2_d.ap())

            # ---- layer 1: hT[m] = gelu(W1_eff @ x^T + b1), (128h, SC s) ----
            hts = []
            acts1 = []
            for m in range(NM1):
                w1m = w1pool.tile([P, NK1 * P], F32R, tag="w1m", name=f"w1m{m}")
                nc.sync.dma_start(out=w1m[:], in_=a1[m])
                hm = hpool.tile([P, SC], F32R, tag=f"h{m}", name=f"h{m}")
                for si in range(NS):
                    ps = pspool.tile([P, NW], F32, tag=f"ps{si}", name=f"ps{si}_{m}")
                    for k in range(NK1):
                        nc.tensor.matmul(
                            ps[:],
                            lhsT=w1m[:, k * P : (k + 1) * P],
                            rhs=xts[k][:, si * NW : (si + 1) * NW],
                            start=(k == 0),
                            stop=(k == NK1 - 1),
                        )
                    act = nc.scalar.activation(
                        hm[:, si * NW : (si + 1) * NW],
                        ps[:],
                        GELU,
                        bias=b1s[:, m : m + 1],
                    )
                acts1.append(act.ins)
                hts.append(hm)

            # ---- layer 2: outT[m] = W2_eff @ gelu_hT + b2, (128o, SC s) ----
            KC = 8          # k-tiles per streamed weight chunk
            for m in range(NM2):
                qss = [
                    pspool.tile([P, NW], F32, tag=f"q{si}", name=f"q{si}_{m}")
                    for si in range(NS)
                ]
                for kc in range(NK2 // KC):
                    w2c = w2pool.tile(
                        [P, KC * P], F32R, tag="w2c", name=f"w2c{m}_{kc}"
                    )
                    w2dma = nc.sync.dma_start(
                        out=w2c[:], in_=a2[m, :, kc * KC * P : (kc + 1) * KC * P]
                    )
                    # keep layer-2 weight prefetch from stealing HBM bandwidth
                    # during startup: issue only after layer-1 is underway
                    gate = min(2 * (m * (NK2 // KC) + kc), NM1 - 1)
                    add_dep_helper(
                        w2dma.ins, acts1[gate], reason="throttle w2 prefetch"
                    )
                    for kk in range(KC):
                        k = kc * KC + kk
                        lhsT = w2c[:, kk * P : (kk + 1) * P]
                        for si in range(NS):
                            nc.tensor.matmul(
                                qss[si][:],
                                lhsT=lhsT,
                                rhs=hts[k][:, si * NW : (si + 1) * NW],
                                start=(k == 0),
                                stop=(k == NK2 - 1),
                            )
                for si in range(NS):
                    om = opool.tile([P, NW], F32, tag=f"o{si}", name=f"o{si}_{m}")
                    nc.scalar.activation(
                        om[:], qss[si][:], IDENT, bias=b2s[:, m : m + 1]
                    )
                    nc.sync.dma_start(
                        out=out[m * P : (m + 1) * P, si * NW : (si + 1) * NW],
                        in_=om[:],
                    )

    _legalize_sync_waits(nc)
    return nc


def prepare_core_inputs(x, W1, b1, W2, b2, scale):
    """Host-side: fold LoRA into weights, tile for the PE, shard per core."""
    x = np.asarray(x, dtype=np.float32)
    W1 = np.asarray(W1, dtype=np.float32)
    W2 = np.asarray(W2, dtype=np.float32)
    b1 = np.asarray(b1, dtype=np.float32)
    b2 = np.asarray(b2, dtype=np.float32)
    scale = np.float32(np.asarray(scale).reshape(()))

    lt = x[:, LORA_TOKEN_INDEX : LORA_TOKEN_INDEX + LORA_DIM, :]
    even = lt[:, ::2, :]  # (B, 8, D)
    odd = lt[:, 1::2, :]  # (B, 8, D)
    m1 = (even[:, :4, :, None] * odd[:, :4, None, :]).reshape(B, H, D)
    m2 = (even[:, 4:, :, None] * odd[:, 4:, None, :]).reshape(B, D, H)
    W1e = W1[None, :, :] + scale * m1  # (B, H, D)
    W2e = W2[None, :, :] + scale * m2  # (B, D, H)

    # lhsT tiles, DMA-contiguous: A1[b][m,p,k,j] = W1e[b, m*128+j, k*128+p]
    A1 = np.ascontiguousarray(
        W1e.reshape(B, NM1, P, NK1, P).transpose(0, 1, 4, 3, 2)
    ).reshape(B, NM1, P, NK1 * P)
    A2 = np.ascontiguousarray(
        W2e.reshape(B, NM2, P, NK2, P).transpose(0, 1, 4, 3, 2)
    ).reshape(B, NM2, P, NK2 * P)
    b1t = np.ascontiguousarray(b1.reshape(NM1, P).T)  # (128, 32)
    b2t = np.ascontiguousarray(b2.reshape(NM2, P).T)  # (128, 8)

    in_maps = []
    for c in range(N_CORES):
        b, r = divmod(c, 2)
        xs = x[b, r * SC : (r + 1) * SC, :]  # (SC, D)
        in_maps.append(
            {
                "xt": np.ascontiguousarray(xs.T),  # (D, SC)
                "a1": A1[b],
                "a2": A2[b],
                "b1t": b1t,
                "b2t": b2t,
            }
        )
    return in_maps


_PROGRAM_CACHE = {}


def get_program():
    if "nc" not in _PROGRAM_CACHE:
        _PROGRAM_CACHE["nc"] = build_program()
    return _PROGRAM_CACHE["nc"]


def kernel(x, W1, b1, W2, b2, scale, _run_kwargs=None):
    nc = get_program()
    in_maps = prepare_core_inputs(x, W1, b1, W2, b2, scale)
    res = run_bass_kernel_spmd(
        nc, in_maps, list(range(N_CORES)), **(_run_kwargs or {})
    )
    out = np.empty((B, S, D), dtype=np.float32)
    for c in range(N_CORES):
        b, r = divmod(c, 2)
        out[b, r * SC : (r + 1) * SC, :] = res.results[c]["out"].T
    if _run_kwargs:
        kernel.last_result = res
    return out
